# revision 19
# baseline (speedup 1.0000x reference)
"""Trainium2 Bass kernel for nn_CDFL1HistogramLoss (CDF-L1 histogram loss).

Math (derived from the reference):
  1. jax.image.resize(bilinear, 512->256, antialiased) is a separable 4-tap
     filter: interior out[i] = (x[2i-1] + 3x[2i] + 3x[2i+1] + x[2i+2])/8,
     edges [3,3,1]/7.  Vertical via PE matmul against a constant 512x256
     band matrix (pre-scaled so downstream values are in u/16 units, and
     streamed in even/odd-split column order so the horizontal filter runs
     on contiguous bf16 slices at DVE 2x rate).
  2. The loss needs T(t) = sum_p sigmoid(c*(u_p - t)) for t = 0..256,
     u = 256*x_resized, c = SIGMA/256.  Bucket pixels by h = round(u/16)
     (17 buckets); within a bucket, sigmoid(c*(16m + 8*wn - t)) as a
     function of the normalized offset wn in [-1,1] is approximated to
     ~6e-3 by a degree-14 polynomial (pole of sigmoid at pi*i/c limits the
     Chebyshev rate; deg 14 suffices).  The approximation error is a fixed
     smooth function of u, so it cancels between the pred and target CDFs.
  3. Device computes per-bucket sums of 15 bounded basis polynomials
     B = {1, x, y..y^7, xy..xy^6}, y = 2x^2-1 (all values in [-1,1]; evens
     are Chebyshev T_2k, so bf16 storage stays well-conditioned):
     A[m, d] = sum_{p in bucket m} B_d(wn_p), via a one-hot matmul scatter
     (one-hot over buckets = weights, basis columns = rhs), PSUM-accumulated
     in 4 column-strips (tile_position) over the 512 pixel columns.
  4. Host maps A -> CDF numerators with an f64 least-squares-fit linear map
     and averages the 48 channel losses.

Sharding: data-parallel over batch N: core i handles batches [2i, 2i+1] of
both pred and target (12 channel-histograms, 6 pred/target pairs per core).
"""
import os
import numpy as np

import concourse.bass as bass
import concourse.bacc as bacc
import concourse.mybir as mybir
from concourse import tile
from concourse.bass_utils import run_bass_kernel_spmd

F32 = mybir.dt.float32
BF16 = mybir.dt.bfloat16
I32 = mybir.dt.int32
ALU = mybir.AluOpType
ACT = mybir.ActivationFunctionType

N_CORES = 8
BINS = 256
SIGMA = 300.0
C = SIGMA / BINS          # 1.171875
N_M = 17                  # coarse buckets h = round(u/16) in [0, 16]
NB = 15                   # basis columns {1, x, y..y^7, x*y..x*y^6}


def make_mh(scale: float = 1.0) -> np.ndarray:
    """[512, 256] vertical resize matrix (jax bilinear antialiased 2x down)."""
    M = np.zeros((512, 256), dtype=np.float64)
    for i in range(256):
        if i == 0:
            M[0, 0], M[1, 0], M[2, 0] = 3 / 7, 3 / 7, 1 / 7
        elif i == 255:
            M[509, 255], M[510, 255], M[511, 255] = 1 / 7, 3 / 7, 3 / 7
        else:
            M[2 * i - 1, i] = 1 / 8
            M[2 * i, i] = 3 / 8
            M[2 * i + 1, i] = 3 / 8
            M[2 * i + 2, i] = 1 / 8
    return (scale * M).astype(np.float32)


def _basis_rows(w: np.ndarray) -> np.ndarray:
    """Device basis values at offsets w in [-1,1]: [..., NB] f64."""
    y = 2.0 * w * w - 1.0
    cols = [np.ones_like(w), w, y, y**2, y**3, y**4, y**5, y**6, y**7,
            w * y, w * y**2, w * y**3, w * y**4, w * y**5, w * y**6]
    return np.stack(cols, axis=-1)


def make_r3c() -> np.ndarray:
    """R3C[m, d, k]: maps bucket-basis sums A[m, d] to CDF numerators
    C[k] = T(0) - T(k+1), k = 0..255, via f64 least-squares fits of
    sigmoid(c*(16m + 8w - t)) in the device basis over w in [-1, 1]."""
    wg = np.cos(np.pi * (np.arange(400) + 0.5) / 400)
    B = _basis_rows(wg)                       # [400, NB]
    tg = np.arange(257.0)
    R3 = np.zeros((N_M, NB, 257))
    for m in range(N_M):
        f = 1.0 / (1.0 + np.exp(-C * (16.0 * m + 8.0 * wg[:, None] - tg[None, :])))
        cf, *_ = np.linalg.lstsq(B, f, rcond=None)   # [NB, 257]
        R3[m] = cf
    return R3[:, :, 0:1] - R3[:, :, 1:257]    # [N_M, NB, 256]


def _nonzero_blocks(MH):
    """Which (half, q) 128x128 blocks of MH are nonzero."""
    blocks = {}
    for half in range(2):
        qs = []
        for q in range(4):
            blk = MH[128 * q:128 * (q + 1), 128 * half:128 * (half + 1)]
            if np.any(blk != 0):
                qs.append(q)
        blocks[half] = qs
    return blocks


def build(n_pairs: int = 6):
    """Build the per-core Bass program. Channels: n_pairs pred + n_pairs target."""
    MHs = make_mh(2.0)               # hs = 2*v so that up = 3*s + t is u/16
    mh_blocks = _nonzero_blocks(MHs)
    n_ch = 2 * n_pairs

    nc = bacc.Bacc("TRN2", target_bir_lowering=False, debug=False, num_devices=N_CORES)
    pred = nc.dram_tensor("pred", [2, 3, 512, 512], F32, kind="ExternalInput").ap()
    target = nc.dram_tensor("target", [2, 3, 512, 512], F32, kind="ExternalInput").ap()
    mh = nc.dram_tensor("mh", [512, 256], F32, kind="ExternalInput").ap()
    out = nc.dram_tensor("out", [128, n_ch * NB], F32, kind="ExternalOutput").ap()

    with tile.TileContext(nc) as tc:
        from contextlib import ExitStack
        nv = nc.vector
        ns = nc.scalar
        ctx = ExitStack()
        cpool = ctx.enter_context(tc.tile_pool(name="consts", bufs=1))

        mh_sb = cpool.tile(shape=[128, 4, 256], dtype=F32, name="mh_sb")
        nc.sync.dma_start(mh_sb, mh.rearrange("(q p) w -> p q w", p=128))
        out_sb = cpool.tile(shape=[128, n_ch, NB], dtype=F32, name="out_sb")

        ch_ctx = ExitStack()
        io_pool = ch_ctx.enter_context(tc.tile_pool(name="io", bufs=2))
        hp_pool = ch_ctx.enter_context(tc.tile_pool(name="hp", bufs=2, space="PSUM"))
        wk_pool = ch_ctx.enter_context(tc.tile_pool(name="wk", bufs=2))
        oh_pool = ch_ctx.enter_context(tc.tile_pool(name="oh", bufs=2))
        a_pool = ch_ctx.enter_context(tc.tile_pool(name="a", bufs=2, space="PSUM"))

        chans = [("p", pi) for pi in range(n_pairs)] + [("t", pi) for pi in range(n_pairs)]

        def emit_prepare(ci):
            grp, pi = chans[ci]
            b, cch = divmod(pi, 3)
            src = (pred if grp == "p" else target)[b, cch]   # [512, 512] dram
            raw = io_pool.tile(shape=[128, 4, 512], dtype=F32, name="raw")
            nc.sync.dma_start(raw, src.rearrange("(q p) w -> p q w", p=128))

            hs = wk_pool.tile(shape=[128, 2, 512], dtype=BF16, name="hs")
            for half in range(2):
                hp = hp_pool.tile(shape=[128, 512], dtype=F32, space="PSUM", name="hp")
                qs = mh_blocks[half]
                # rhs streamed in even/odd-split order -> hp = [ev(256) | od(256)]
                for qi, q in enumerate(qs):
                    rq = (raw[:, q, :]
                          .rearrange("p (h two) -> p h two", two=2)
                          .rearrange("p h two -> p two h"))
                    nc.tensor.matmul(
                        hp, mh_sb[:, q, 128 * half:128 * (half + 1)], rq,
                        start=(qi == 0), stop=(qi == len(qs) - 1),
                    )
                ns.copy(hs[:, half], hp)

            # horizontal 4-tap in u/16 units: up[i] = 3*(ev[i]+od[i]) + od[i-1]+ev[i+1]
            s = wk_pool.tile(shape=[128, 2, 256], dtype=BF16, name="s")
            t = wk_pool.tile(shape=[128, 2, 256], dtype=BF16, name="t")
            up = wk_pool.tile(shape=[128, 2, 256], dtype=BF16, name="up")
            te = wk_pool.tile(shape=[128, 4], dtype=BF16, name="te")
            for half in range(2):
                ev, od = hs[:, half, 0:256], hs[:, half, 256:512]
                nv.tensor_tensor(s[:, half], ev, od, ALU.add)
                nv.tensor_tensor(t[:, half, 1:255], od[:, 0:254], ev[:, 2:256], ALU.add)
                # edges: up[0] = (24/7)s[0] + (8/7)ev[1]; up[255] = (24/7)s[255] + (8/7)od[254]
                nv.tensor_scalar(te[:, 2 * half:2 * half + 1], ev[:, 1:2], 8.0 / 7.0, None, ALU.mult)
                nv.tensor_scalar(te[:, 2 * half + 1:2 * half + 2], od[:, 254:255], 8.0 / 7.0, None, ALU.mult)
                nv.scalar_tensor_tensor(t[:, half, 0:1], s[:, half, 0:1], 3.0 / 7.0,
                                        te[:, 2 * half:2 * half + 1], ALU.mult, ALU.add)
                nv.scalar_tensor_tensor(t[:, half, 255:256], s[:, half, 255:256], 3.0 / 7.0,
                                        te[:, 2 * half + 1:2 * half + 2], ALU.mult, ALU.add)
                nv.scalar_tensor_tensor(up[:, half], s[:, half], 3.0, t[:, half], ALU.mult, ALU.add)

            upf = up.rearrange("p h i -> p (h i)")          # [128, 512], u/16 in [0,16]
            # h = round(up) via bf16 magic constant: up+128 lands on the
            # integer grid (ulp=1 in [128,256)), rounding to nearest int.
            hr = wk_pool.tile(shape=[128, 512], dtype=BF16, name="hr")
            nv.tensor_scalar(hr, upf, 128.0, None, ALU.add)
            h2 = wk_pool.tile(shape=[128, 512], dtype=BF16, name="h2")
            nv.tensor_scalar(h2, hr, 2.0, -256.0, ALU.mult, ALU.add)  # 2h, exact

            V = oh_pool.tile(shape=[128, NB, 512], dtype=BF16, name="V")
            if ci < 2:
                nc.gpsimd.memset(V[:, 0, :], 1.0)           # ones plane, per physical buf
            x = V[:, 1, :]
            nv.scalar_tensor_tensor(x, upf, 2.0, h2, ALU.mult, ALU.subtract)  # wn in [-1,1]
            wnsq = wk_pool.tile(shape=[128, 512], dtype=BF16, name="wnsq")
            ns.activation(wnsq, x, ACT.Square)
            y = V[:, 2, :]
            nv.tensor_scalar(y, wnsq, 2.0, -1.0, ALU.mult, ALU.add)
            ns.activation(V[:, 3, :], y, ACT.Square)                      # y2
            nv.tensor_tensor(V[:, 4, :], V[:, 3, :], y, ALU.mult)         # y3
            ns.activation(V[:, 5, :], V[:, 3, :], ACT.Square)             # y4
            nv.tensor_tensor(V[:, 6, :], V[:, 5, :], y, ALU.mult)         # y5
            nv.tensor_tensor(V[:, 7, :], V[:, 5, :], V[:, 3, :], ALU.mult)  # y6
            nv.tensor_tensor(V[:, 8, :], V[:, 5, :], V[:, 4, :], ALU.mult)  # y7
            nv.tensor_tensor(V[:, 9, :], x, y, ALU.mult)                  # xy
            nv.tensor_tensor(V[:, 10, :], x, V[:, 3, :], ALU.mult)        # xy2
            nv.tensor_tensor(V[:, 11, :], V[:, 10, :], y, ALU.mult)       # xy3
            nv.tensor_tensor(V[:, 12, :], x, V[:, 5, :], ALU.mult)        # xy4
            nv.tensor_tensor(V[:, 13, :], V[:, 12, :], y, ALU.mult)       # xy5
            nv.tensor_tensor(V[:, 14, :], V[:, 12, :], V[:, 3, :], ALU.mult)  # xy6

            OH = oh_pool.tile(shape=[128, N_M, 512], dtype=BF16, name="OH")
            for m in range(N_M):
                nv.tensor_scalar(OH[:, m, :], h2, float(2 * m), None, ALU.is_equal)
            return (ci, OH, V)

        def emit_scatter(st):
            ci, OH, V = st
            G = 4
            a_ps = a_pool.tile(shape=[128, 512], dtype=F32, space="PSUM", name="a_ps")
            for f in range(512):
                g = f % G
                nc.tensor.matmul(a_ps[32 * g:32 * g + N_M, 0:NB], OH[:, :, f], V[:, :, f],
                                 start=(f < G), stop=(f >= 512 - G),
                                 tile_position=(0, 32 * g), skip_group_check=True)
            return (ci, a_ps)

        # Skewed emission: prepare(ci) ahead of scatter(ci-1); the PSUM->SBUF
        # result copy is deferred one further iteration so its scatter-done
        # wait never head-of-line-blocks prepare work in the ScalarE FIFO.
        state = None
        pend = None
        for ci in range(n_ch + 1):
            nxt = emit_prepare(ci) if ci < n_ch else None
            if pend is not None:
                ns.copy(out_sb[:, pend[0], :], pend[1][:, 0:NB])
            if state is not None:
                pend = emit_scatter(state)
            state = nxt
        ns.copy(out_sb[:, pend[0], :], pend[1][:, 0:NB])

        ch_ctx.close()
        nc.sync.dma_start(out, out_sb.rearrange("p c d -> p (c d)"))
        ctx.close()

    nc.compile()
    return nc


_CACHE: dict = {}
LAST_RESULT = None


def _get_nc(n_pairs=6):
    key = n_pairs
    if key not in _CACHE:
        _CACHE[key] = build(n_pairs)
    return _CACHE[key]


def kernel(pred: np.ndarray, target: np.ndarray) -> np.ndarray:
    global LAST_RESULT
    pred = np.ascontiguousarray(pred, dtype=np.float32)
    target = np.ascontiguousarray(target, dtype=np.float32)
    assert pred.shape == (16, 3, 512, 512) and target.shape == (16, 3, 512, 512)

    nc = _get_nc(6)
    mh_buf = make_mh(2.0)
    in_maps = []
    for i in range(N_CORES):
        in_maps.append({
            "pred": pred[2 * i:2 * i + 2],
            "target": target[2 * i:2 * i + 2],
            "mh": mh_buf,
        })
    trace = os.environ.get("KERNEL_TRACE", "0") == "1"
    res = run_bass_kernel_spmd(nc, in_maps, core_ids=list(range(N_CORES)), trace=trace)
    LAST_RESULT = res

    R3C = make_r3c().reshape(N_M * NB, 256)         # [(m,d), k] f64
    losses = []
    for i in range(N_CORES):
        raw = res.results[i]["out"].astype(np.float64).reshape(128, 12, NB)
        # combine the 4 tile_position column-strips: A[ch, m, d]
        A = sum(raw[32 * g:32 * g + N_M] for g in range(4))   # [N_M, 12, NB]
        A = A.transpose(1, 0, 2).reshape(12, N_M * NB)
        Cn = A @ R3C                                          # [12, 256]
        for p in range(6):
            Cp, Ct = Cn[p], Cn[p + 6]
            losses.append(np.mean(np.abs(Cp / Cp[-1] - Ct / Ct[-1])))
    return np.float32(np.mean(losses))


# revision 31
# speedup vs baseline: 1.0058x; 1.0058x over previous
"""Trainium2 Bass kernel for nn_CDFL1HistogramLoss (CDF-L1 histogram loss).

Math (derived from the reference):
  1. jax.image.resize(bilinear, 512->256, antialiased) is a separable 4-tap
     filter: interior out[i] = (x[2i-1] + 3x[2i] + 3x[2i+1] + x[2i+2])/8,
     edges [3,3,1]/7.  Vertical via PE matmul against a constant 512x256
     band matrix (pre-scaled so downstream values are in u/16 units, and
     streamed in even/odd-split column order so the horizontal filter runs
     on contiguous bf16 slices at DVE 2x rate).
  2. The loss needs T(t) = sum_p sigmoid(c*(u_p - t)) for t = 0..256,
     u = 256*x_resized, c = SIGMA/256.  Bucket pixels by h = round(u/16)
     (17 buckets); within a bucket, sigmoid(c*(16m + 8*wn - t)) as a
     function of the normalized offset wn in [-1,1] is approximated to
     ~6e-3 by a degree-14 polynomial (pole of sigmoid at pi*i/c limits the
     Chebyshev rate; deg 14 suffices).  The approximation error is a fixed
     smooth function of u, so it cancels between the pred and target CDFs.
  3. Device computes per-bucket sums of 15 bounded basis polynomials
     B = {1, x, y..y^7, xy..xy^6}, y = 2x^2-1 (all values in [-1,1]; evens
     are Chebyshev T_2k, so bf16 storage stays well-conditioned):
     A[m, d] = sum_{p in bucket m} B_d(wn_p), via a one-hot matmul scatter
     (one-hot over buckets = weights, basis columns = rhs), PSUM-accumulated
     in 4 column-strips (tile_position) over the 512 pixel columns.
  4. Host maps A -> CDF numerators with an f64 least-squares-fit linear map
     and averages the 48 channel losses.

Sharding: data-parallel over batch N: core i handles batches [2i, 2i+1] of
both pred and target (12 channel-histograms, 6 pred/target pairs per core).
"""
import os
import numpy as np

import concourse.bass as bass
import concourse.bacc as bacc
import concourse.mybir as mybir
from concourse import tile
from concourse.bass_utils import run_bass_kernel_spmd

F32 = mybir.dt.float32
BF16 = mybir.dt.bfloat16
I32 = mybir.dt.int32
ALU = mybir.AluOpType
ACT = mybir.ActivationFunctionType

N_CORES = 8
BINS = 256
SIGMA = 300.0
C = SIGMA / BINS          # 1.171875
N_M = 17                  # coarse buckets h = round(u/16) in [0, 16]
NB = 15                   # basis columns {1, x, y..y^7, x*y..x*y^6}


def make_mh(scale: float = 1.0) -> np.ndarray:
    """[512, 256] vertical resize matrix (jax bilinear antialiased 2x down)."""
    M = np.zeros((512, 256), dtype=np.float64)
    for i in range(256):
        if i == 0:
            M[0, 0], M[1, 0], M[2, 0] = 3 / 7, 3 / 7, 1 / 7
        elif i == 255:
            M[509, 255], M[510, 255], M[511, 255] = 1 / 7, 3 / 7, 3 / 7
        else:
            M[2 * i - 1, i] = 1 / 8
            M[2 * i, i] = 3 / 8
            M[2 * i + 1, i] = 3 / 8
            M[2 * i + 2, i] = 1 / 8
    return (scale * M).astype(np.float32)


def _basis_rows(w: np.ndarray) -> np.ndarray:
    """Device basis values at offsets w in [-1,1]: [..., NB] f64."""
    y = 2.0 * w * w - 1.0
    cols = [np.ones_like(w), w, y, y**2, y**3, y**4, y**5, y**6, y**7,
            w * y, w * y**2, w * y**3, w * y**4, w * y**5, w * y**6]
    return np.stack(cols, axis=-1)


def make_r3c() -> np.ndarray:
    """R3C[m, d, k]: maps bucket-basis sums A[m, d] to CDF numerators
    C[k] = T(0) - T(k+1), k = 0..255, via f64 least-squares fits of
    sigmoid(c*(16m + 8w - t)) in the device basis over w in [-1, 1]."""
    wg = np.cos(np.pi * (np.arange(400) + 0.5) / 400)
    B = _basis_rows(wg)                       # [400, NB]
    tg = np.arange(257.0)
    R3 = np.zeros((N_M, NB, 257))
    for m in range(N_M):
        f = 1.0 / (1.0 + np.exp(-C * (16.0 * m + 8.0 * wg[:, None] - tg[None, :])))
        cf, *_ = np.linalg.lstsq(B, f, rcond=None)   # [NB, 257]
        R3[m] = cf
    return R3[:, :, 0:1] - R3[:, :, 1:257]    # [N_M, NB, 256]


def _nonzero_blocks(MH):
    """Which (half, q) 128x128 blocks of MH are nonzero."""
    blocks = {}
    for half in range(2):
        qs = []
        for q in range(4):
            blk = MH[128 * q:128 * (q + 1), 128 * half:128 * (half + 1)]
            if np.any(blk != 0):
                qs.append(q)
        blocks[half] = qs
    return blocks


def build(n_pairs: int = 6):
    """Build the per-core Bass program. Channels: n_pairs pred + n_pairs target."""
    MHs = make_mh(2.0)               # hs = 2*v so that up = 3*s + t is u/16
    mh_blocks = _nonzero_blocks(MHs)
    n_ch = 2 * n_pairs

    nc = bacc.Bacc("TRN2", target_bir_lowering=False, debug=False, num_devices=N_CORES)
    pred = nc.dram_tensor("pred", [2, 3, 512, 512], F32, kind="ExternalInput").ap()
    target = nc.dram_tensor("target", [2, 3, 512, 512], F32, kind="ExternalInput").ap()
    mh = nc.dram_tensor("mh", [512, 256], F32, kind="ExternalInput").ap()
    out = nc.dram_tensor("out", [128, n_ch * NB], F32, kind="ExternalOutput").ap()

    with tile.TileContext(nc) as tc:
        from contextlib import ExitStack
        nv = nc.vector
        ns = nc.scalar
        ctx = ExitStack()
        cpool = ctx.enter_context(tc.tile_pool(name="consts", bufs=1))

        mh_sb = cpool.tile(shape=[128, 4, 256], dtype=F32, name="mh_sb")
        nc.sync.dma_start(mh_sb, mh.rearrange("(q p) w -> p q w", p=128))
        out_sb = cpool.tile(shape=[128, n_ch, NB], dtype=F32, name="out_sb")

        ch_ctx = ExitStack()
        io_pool = ch_ctx.enter_context(tc.tile_pool(name="io", bufs=2))
        hp_pool = ch_ctx.enter_context(tc.tile_pool(name="hp", bufs=2, space="PSUM"))
        wk_pool = ch_ctx.enter_context(tc.tile_pool(name="wk", bufs=2))
        oh_pool = ch_ctx.enter_context(tc.tile_pool(name="oh", bufs=2))
        a_pool = ch_ctx.enter_context(tc.tile_pool(name="a", bufs=2, space="PSUM"))

        chans = [("p", pi) for pi in range(n_pairs)] + [("t", pi) for pi in range(n_pairs)]

        def emit_prepare(ci):
            grp, pi = chans[ci]
            b, cch = divmod(pi, 3)
            src = (pred if grp == "p" else target)[b, cch]   # [512, 512] dram
            raw = io_pool.tile(shape=[128, 4, 512], dtype=F32, name="raw")
            srcq = src.rearrange("(q p) w -> q p w", p=128)
            for q in range(4):      # per-q chunks so resize starts sooner
                nc.sync.dma_start(raw[:, q, :], srcq[q])

            hs = wk_pool.tile(shape=[128, 2, 512], dtype=BF16, name="hs")
            for half in range(2):
                hp = hp_pool.tile(shape=[128, 512], dtype=F32, space="PSUM", name="hp")
                qs = mh_blocks[half]
                # rhs streamed in even/odd-split order -> hp = [ev(256) | od(256)]
                for qi, q in enumerate(qs):
                    rq = (raw[:, q, :]
                          .rearrange("p (h two) -> p h two", two=2)
                          .rearrange("p h two -> p two h"))
                    nc.tensor.matmul(
                        hp, mh_sb[:, q, 128 * half:128 * (half + 1)], rq,
                        start=(qi == 0), stop=(qi == len(qs) - 1),
                    )
                nv.tensor_copy(hs[:, half], hp)

            # horizontal 4-tap in u/16 units: up[i] = 3*(ev[i]+od[i]) + od[i-1]+ev[i+1]
            s = wk_pool.tile(shape=[128, 2, 256], dtype=BF16, name="s")
            t = wk_pool.tile(shape=[128, 2, 256], dtype=BF16, name="t")
            up = wk_pool.tile(shape=[128, 2, 256], dtype=BF16, name="up")
            te = wk_pool.tile(shape=[128, 4], dtype=BF16, name="te")
            for half in range(2):
                ev, od = hs[:, half, 0:256], hs[:, half, 256:512]
                nv.tensor_tensor(s[:, half], ev, od, ALU.add)
                nv.tensor_tensor(t[:, half, 1:255], od[:, 0:254], ev[:, 2:256], ALU.add)
                # edges: up[0] = (24/7)s[0] + (8/7)ev[1]; up[255] = (24/7)s[255] + (8/7)od[254]
                nv.tensor_scalar(te[:, 2 * half:2 * half + 1], ev[:, 1:2], 8.0 / 7.0, None, ALU.mult)
                nv.tensor_scalar(te[:, 2 * half + 1:2 * half + 2], od[:, 254:255], 8.0 / 7.0, None, ALU.mult)
                nv.scalar_tensor_tensor(t[:, half, 0:1], s[:, half, 0:1], 3.0 / 7.0,
                                        te[:, 2 * half:2 * half + 1], ALU.mult, ALU.add)
                nv.scalar_tensor_tensor(t[:, half, 255:256], s[:, half, 255:256], 3.0 / 7.0,
                                        te[:, 2 * half + 1:2 * half + 2], ALU.mult, ALU.add)
                nv.scalar_tensor_tensor(up[:, half], s[:, half], 3.0, t[:, half], ALU.mult, ALU.add)

            upf = up.rearrange("p h i -> p (h i)")          # [128, 512], u/16 in [0,16]
            # h = round(up) via bf16 magic constant: up+128 lands on the
            # integer grid (ulp=1 in [128,256)), rounding to nearest int.
            hr = wk_pool.tile(shape=[128, 512], dtype=BF16, name="hr")
            nv.tensor_scalar(hr, upf, 128.0, None, ALU.add)
            h2 = wk_pool.tile(shape=[128, 512], dtype=BF16, name="h2")
            nv.tensor_scalar(h2, hr, 2.0, -256.0, ALU.mult, ALU.add)  # 2h, exact

            V = oh_pool.tile(shape=[128, NB, 512], dtype=BF16, name="V")
            if ci < 2:
                nc.gpsimd.memset(V[:, 0, :], 1.0)           # ones plane, per physical buf
            x = V[:, 1, :]
            nv.scalar_tensor_tensor(x, upf, 2.0, h2, ALU.mult, ALU.subtract)  # wn in [-1,1]
            wnsq = wk_pool.tile(shape=[128, 512], dtype=BF16, name="wnsq")
            nv.tensor_tensor(wnsq, x, x, ALU.mult)
            y = V[:, 2, :]
            nv.tensor_scalar(y, wnsq, 2.0, -1.0, ALU.mult, ALU.add)
            nv.tensor_tensor(V[:, 3, :], y, y, ALU.mult)                  # y2
            nv.tensor_tensor(V[:, 4, :], V[:, 3, :], y, ALU.mult)         # y3
            nv.tensor_tensor(V[:, 5, :], V[:, 3, :], V[:, 3, :], ALU.mult)  # y4
            nv.tensor_tensor(V[:, 6, :], V[:, 5, :], y, ALU.mult)         # y5
            nv.tensor_tensor(V[:, 7, :], V[:, 5, :], V[:, 3, :], ALU.mult)  # y6
            nv.tensor_tensor(V[:, 8, :], V[:, 5, :], V[:, 4, :], ALU.mult)  # y7
            nv.tensor_tensor(V[:, 9, :], x, y, ALU.mult)                  # xy
            nv.tensor_tensor(V[:, 10, :], x, V[:, 3, :], ALU.mult)        # xy2
            nv.tensor_tensor(V[:, 11, :], V[:, 10, :], y, ALU.mult)       # xy3
            nv.tensor_tensor(V[:, 12, :], x, V[:, 5, :], ALU.mult)        # xy4
            nv.tensor_tensor(V[:, 13, :], V[:, 12, :], y, ALU.mult)       # xy5
            nv.tensor_tensor(V[:, 14, :], V[:, 12, :], V[:, 3, :], ALU.mult)  # xy6

            OH = oh_pool.tile(shape=[128, N_M, 512], dtype=BF16, name="OH")
            for m in range(N_M):
                nv.tensor_scalar(OH[:, m, :], h2, float(2 * m), None, ALU.is_equal)
            return (ci, OH, V)

        def emit_scatter(st):
            ci, OH, V = st
            G = 4
            a_ps = a_pool.tile(shape=[128, 512], dtype=F32, space="PSUM", name="a_ps")
            for f in range(512):
                g = f % G
                nc.tensor.matmul(a_ps[32 * g:32 * g + N_M, 0:NB], OH[:, :, f], V[:, :, f],
                                 start=(f < G), stop=(f >= 512 - G),
                                 tile_position=(0, 32 * g), skip_group_check=True)
            # ScalarE runs ONLY these copies, so its scatter-done wait cannot
            # head-of-line-block any prepare work
            ns.copy(out_sb[:, ci, :], a_ps[:, 0:NB])

        state = None
        for ci in range(n_ch + 1):
            nxt = emit_prepare(ci) if ci < n_ch else None
            if state is not None:
                emit_scatter(state)
            state = nxt

        ch_ctx.close()
        nc.sync.dma_start(out, out_sb.rearrange("p c d -> p (c d)"))
        ctx.close()

    nc.compile()
    return nc


_CACHE: dict = {}
LAST_RESULT = None


def _get_nc(n_pairs=6):
    key = n_pairs
    if key not in _CACHE:
        _CACHE[key] = build(n_pairs)
    return _CACHE[key]


def kernel(pred: np.ndarray, target: np.ndarray) -> np.ndarray:
    global LAST_RESULT
    pred = np.ascontiguousarray(pred, dtype=np.float32)
    target = np.ascontiguousarray(target, dtype=np.float32)
    assert pred.shape == (16, 3, 512, 512) and target.shape == (16, 3, 512, 512)

    nc = _get_nc(6)
    mh_buf = make_mh(2.0)
    in_maps = []
    for i in range(N_CORES):
        in_maps.append({
            "pred": pred[2 * i:2 * i + 2],
            "target": target[2 * i:2 * i + 2],
            "mh": mh_buf,
        })
    trace = os.environ.get("KERNEL_TRACE", "0") == "1"
    res = run_bass_kernel_spmd(nc, in_maps, core_ids=list(range(N_CORES)), trace=trace)
    LAST_RESULT = res

    R3C = make_r3c().reshape(N_M * NB, 256)         # [(m,d), k] f64
    losses = []
    for i in range(N_CORES):
        raw = res.results[i]["out"].astype(np.float64).reshape(128, 12, NB)
        # combine the 4 tile_position column-strips: A[ch, m, d]
        A = sum(raw[32 * g:32 * g + N_M] for g in range(4))   # [N_M, 12, NB]
        A = A.transpose(1, 0, 2).reshape(12, N_M * NB)
        Cn = A @ R3C                                          # [12, 256]
        for p in range(6):
            Cp, Ct = Cn[p], Cn[p + 6]
            losses.append(np.mean(np.abs(Cp / Cp[-1] - Ct / Ct[-1])))
    return np.float32(np.mean(losses))


# revision 33
# speedup vs baseline: 1.1693x; 1.1626x over previous
"""Trainium2 Bass kernel for nn_CDFL1HistogramLoss (CDF-L1 histogram loss).

Math (derived from the reference):
  1. jax.image.resize(bilinear, 512->256, antialiased) is a separable 4-tap
     filter: interior out[i] = (x[2i-1] + 3x[2i] + 3x[2i+1] + x[2i+2])/8,
     edges [3,3,1]/7.  Vertical via PE matmul against a constant 512x256
     band matrix (pre-scaled so downstream values are in u/16 units, and
     streamed in even/odd-split column order so the horizontal filter runs
     on contiguous bf16 slices at DVE 2x rate).
  2. The loss needs T(t) = sum_p sigmoid(c*(u_p - t)) for t = 0..256,
     u = 256*x_resized, c = SIGMA/256.  Bucket pixels by h = round(u/16)
     (17 buckets); within a bucket, sigmoid(c*(16m + 8*wn - t)) as a
     function of the normalized offset wn in [-1,1] is approximated to
     ~6e-3 by a degree-14 polynomial (pole of sigmoid at pi*i/c limits the
     Chebyshev rate; deg 14 suffices).  The approximation error is a fixed
     smooth function of u, so it cancels between the pred and target CDFs.
  3. Device computes per-bucket sums of 15 bounded basis polynomials
     B = {1, x, y..y^7, xy..xy^6}, y = 2x^2-1 (all values in [-1,1]; evens
     are Chebyshev T_2k, so bf16 storage stays well-conditioned):
     A[m, d] = sum_{p in bucket m} B_d(wn_p), via a one-hot matmul scatter
     (one-hot over buckets = weights, basis columns = rhs), PSUM-accumulated
     in 4 column-strips (tile_position) over the 512 pixel columns.
  4. Host maps A -> CDF numerators with an f64 least-squares-fit linear map
     and averages the 48 channel losses.

Sharding: data-parallel over batch N: core i handles batches [2i, 2i+1] of
both pred and target (12 channel-histograms, 6 pred/target pairs per core).
"""
import os
import numpy as np

import concourse.bass as bass
import concourse.bacc as bacc
import concourse.mybir as mybir
from concourse import tile
from concourse.bass_utils import run_bass_kernel_spmd

F32 = mybir.dt.float32
BF16 = mybir.dt.bfloat16
I32 = mybir.dt.int32
ALU = mybir.AluOpType
ACT = mybir.ActivationFunctionType

N_CORES = 8
BINS = 256
SIGMA = 300.0
C = SIGMA / BINS          # 1.171875
N_M = 17                  # coarse buckets h = round(u/16) in [0, 16]
NB = 15                   # basis columns {1, x, y..y^7, x*y..x*y^6}


def make_mh(scale: float = 1.0) -> np.ndarray:
    """[512, 256] vertical resize matrix (jax bilinear antialiased 2x down)."""
    M = np.zeros((512, 256), dtype=np.float64)
    for i in range(256):
        if i == 0:
            M[0, 0], M[1, 0], M[2, 0] = 3 / 7, 3 / 7, 1 / 7
        elif i == 255:
            M[509, 255], M[510, 255], M[511, 255] = 1 / 7, 3 / 7, 3 / 7
        else:
            M[2 * i - 1, i] = 1 / 8
            M[2 * i, i] = 3 / 8
            M[2 * i + 1, i] = 3 / 8
            M[2 * i + 2, i] = 1 / 8
    return (scale * M).astype(np.float32)


def _basis_rows(w: np.ndarray) -> np.ndarray:
    """Device basis values at offsets w in [-1,1]: [..., NB] f64."""
    y = 2.0 * w * w - 1.0
    cols = [np.ones_like(w), w, y, y**2, y**3, y**4, y**5, y**6, y**7,
            w * y, w * y**2, w * y**3, w * y**4, w * y**5, w * y**6]
    return np.stack(cols, axis=-1)


def make_r3c() -> np.ndarray:
    """R3C[m, d, k]: maps bucket-basis sums A[m, d] to CDF numerators
    C[k] = T(0) - T(k+1), k = 0..255, via f64 least-squares fits of
    sigmoid(c*(16m + 8w - t)) in the device basis over w in [-1, 1]."""
    wg = np.cos(np.pi * (np.arange(400) + 0.5) / 400)
    B = _basis_rows(wg)                       # [400, NB]
    tg = np.arange(257.0)
    R3 = np.zeros((N_M, NB, 257))
    for m in range(N_M):
        f = 1.0 / (1.0 + np.exp(-C * (16.0 * m + 8.0 * wg[:, None] - tg[None, :])))
        cf, *_ = np.linalg.lstsq(B, f, rcond=None)   # [NB, 257]
        R3[m] = cf
    return R3[:, :, 0:1] - R3[:, :, 1:257]    # [N_M, NB, 256]


def _nonzero_blocks(MH):
    """Which (half, q) 128x128 blocks of MH are nonzero."""
    blocks = {}
    for half in range(2):
        qs = []
        for q in range(4):
            blk = MH[128 * q:128 * (q + 1), 128 * half:128 * (half + 1)]
            if np.any(blk != 0):
                qs.append(q)
        blocks[half] = qs
    return blocks


def build(n_pairs: int = 6):
    """Build the per-core Bass program. Channels: n_pairs pred + n_pairs target."""
    MHs = make_mh(2.0)               # hs = 2*v so that up = 3*s + t is u/16
    mh_blocks = _nonzero_blocks(MHs)
    n_ch = 2 * n_pairs

    nc = bacc.Bacc("TRN2", target_bir_lowering=False, debug=False, num_devices=N_CORES)
    pred = nc.dram_tensor("pred", [2, 3, 512, 512], F32, kind="ExternalInput").ap()
    target = nc.dram_tensor("target", [2, 3, 512, 512], F32, kind="ExternalInput").ap()
    mh = nc.dram_tensor("mh", [512, 256], F32, kind="ExternalInput").ap()
    out = nc.dram_tensor("out", [128, n_ch * NB], F32, kind="ExternalOutput").ap()

    with tile.TileContext(nc) as tc:
        from contextlib import ExitStack
        nv = nc.vector
        ns = nc.scalar
        ctx = ExitStack()
        cpool = ctx.enter_context(tc.tile_pool(name="consts", bufs=1))

        mh_sb = cpool.tile(shape=[128, 4, 256], dtype=F32, name="mh_sb")
        nc.sync.dma_start(mh_sb, mh.rearrange("(q p) w -> p q w", p=128))
        mhb = cpool.tile(shape=[128, 4, 256], dtype=BF16, name="mhb")
        nv.tensor_copy(mhb, mh_sb)
        out_sb = cpool.tile(shape=[128, n_ch, NB], dtype=F32, name="out_sb")

        ch_ctx = ExitStack()
        io_pool = ch_ctx.enter_context(tc.tile_pool(name="io", bufs=2))
        hp_pool = ch_ctx.enter_context(tc.tile_pool(name="hp", bufs=2, space="PSUM"))
        wk_pool = ch_ctx.enter_context(tc.tile_pool(name="wk", bufs=2))
        oh_pool = ch_ctx.enter_context(tc.tile_pool(name="oh", bufs=2))
        a_pool = ch_ctx.enter_context(tc.tile_pool(name="a", bufs=2, space="PSUM"))

        chans = [("p", pi) for pi in range(n_pairs)] + [("t", pi) for pi in range(n_pairs)]

        def emit_prepare(ci):
            grp, pi = chans[ci]
            b, cch = divmod(pi, 3)
            src = (pred if grp == "p" else target)[b, cch]   # [512, 512] dram
            raw = io_pool.tile(shape=[128, 4, 512], dtype=F32, name="raw")
            srcq = src.rearrange("(q p) w -> q p w", p=128)
            for q in range(4):      # per-q chunks so the cast starts sooner
                nc.sync.dma_start(raw[:, q, :], srcq[q])
            rawb = io_pool.tile(shape=[128, 4, 512], dtype=BF16, name="rawb")
            nv.tensor_copy(rawb, raw)   # bf16 weights+rhs enable PE fast weight load

            hs = wk_pool.tile(shape=[128, 2, 512], dtype=BF16, name="hs")
            for half in range(2):
                hp = hp_pool.tile(shape=[128, 512], dtype=F32, space="PSUM", name="hp")
                qs = mh_blocks[half]
                # rhs streamed in even/odd-split order -> hp = [ev(256) | od(256)]
                for qi, q in enumerate(qs):
                    rq = (rawb[:, q, :]
                          .rearrange("p (h two) -> p h two", two=2)
                          .rearrange("p h two -> p two h"))
                    nc.tensor.matmul(
                        hp, mhb[:, q, 128 * half:128 * (half + 1)], rq,
                        start=(qi == 0), stop=(qi == len(qs) - 1),
                    )
                nv.tensor_copy(hs[:, half], hp)

            # horizontal 4-tap in u/16 units: up[i] = 3*(ev[i]+od[i]) + od[i-1]+ev[i+1]
            s = wk_pool.tile(shape=[128, 2, 256], dtype=BF16, name="s")
            t = wk_pool.tile(shape=[128, 2, 256], dtype=BF16, name="t")
            up = wk_pool.tile(shape=[128, 2, 256], dtype=BF16, name="up")
            te = wk_pool.tile(shape=[128, 4], dtype=BF16, name="te")
            for half in range(2):
                ev, od = hs[:, half, 0:256], hs[:, half, 256:512]
                nv.tensor_tensor(s[:, half], ev, od, ALU.add)
                nv.tensor_tensor(t[:, half, 1:255], od[:, 0:254], ev[:, 2:256], ALU.add)
                # edges: up[0] = (24/7)s[0] + (8/7)ev[1]; up[255] = (24/7)s[255] + (8/7)od[254]
                nv.tensor_scalar(te[:, 2 * half:2 * half + 1], ev[:, 1:2], 8.0 / 7.0, None, ALU.mult)
                nv.tensor_scalar(te[:, 2 * half + 1:2 * half + 2], od[:, 254:255], 8.0 / 7.0, None, ALU.mult)
                nv.scalar_tensor_tensor(t[:, half, 0:1], s[:, half, 0:1], 3.0 / 7.0,
                                        te[:, 2 * half:2 * half + 1], ALU.mult, ALU.add)
                nv.scalar_tensor_tensor(t[:, half, 255:256], s[:, half, 255:256], 3.0 / 7.0,
                                        te[:, 2 * half + 1:2 * half + 2], ALU.mult, ALU.add)
                nv.scalar_tensor_tensor(up[:, half], s[:, half], 3.0, t[:, half], ALU.mult, ALU.add)

            upf = up.rearrange("p h i -> p (h i)")          # [128, 512], u/16 in [0,16]
            # h = round(up) via bf16 magic constant: up+128 lands on the
            # integer grid (ulp=1 in [128,256)), rounding to nearest int.
            hr = wk_pool.tile(shape=[128, 512], dtype=BF16, name="hr")
            nv.tensor_scalar(hr, upf, 128.0, None, ALU.add)
            h2 = wk_pool.tile(shape=[128, 512], dtype=BF16, name="h2")
            nv.tensor_scalar(h2, hr, 2.0, -256.0, ALU.mult, ALU.add)  # 2h, exact

            V = oh_pool.tile(shape=[128, NB, 512], dtype=BF16, name="V")
            if ci < 2:
                nc.gpsimd.memset(V[:, 0, :], 1.0)           # ones plane, per physical buf
            x = V[:, 1, :]
            nv.scalar_tensor_tensor(x, upf, 2.0, h2, ALU.mult, ALU.subtract)  # wn in [-1,1]
            wnsq = wk_pool.tile(shape=[128, 512], dtype=BF16, name="wnsq")
            nv.tensor_tensor(wnsq, x, x, ALU.mult)
            y = V[:, 2, :]
            nv.tensor_scalar(y, wnsq, 2.0, -1.0, ALU.mult, ALU.add)
            nv.tensor_tensor(V[:, 3, :], y, y, ALU.mult)                  # y2
            nv.tensor_tensor(V[:, 4, :], V[:, 3, :], y, ALU.mult)         # y3
            nv.tensor_tensor(V[:, 5, :], V[:, 3, :], V[:, 3, :], ALU.mult)  # y4
            nv.tensor_tensor(V[:, 6, :], V[:, 5, :], y, ALU.mult)         # y5
            nv.tensor_tensor(V[:, 7, :], V[:, 5, :], V[:, 3, :], ALU.mult)  # y6
            nv.tensor_tensor(V[:, 8, :], V[:, 5, :], V[:, 4, :], ALU.mult)  # y7
            nv.tensor_tensor(V[:, 9, :], x, y, ALU.mult)                  # xy
            nv.tensor_tensor(V[:, 10, :], x, V[:, 3, :], ALU.mult)        # xy2
            nv.tensor_tensor(V[:, 11, :], V[:, 10, :], y, ALU.mult)       # xy3
            nv.tensor_tensor(V[:, 12, :], x, V[:, 5, :], ALU.mult)        # xy4
            nv.tensor_tensor(V[:, 13, :], V[:, 12, :], y, ALU.mult)       # xy5
            nv.tensor_tensor(V[:, 14, :], V[:, 12, :], V[:, 3, :], ALU.mult)  # xy6

            OH = oh_pool.tile(shape=[128, N_M, 512], dtype=BF16, name="OH")
            for m in range(N_M):
                nv.tensor_scalar(OH[:, m, :], h2, float(2 * m), None, ALU.is_equal)
            return (ci, OH, V)

        def emit_scatter(st):
            ci, OH, V = st
            G = 4
            a_ps = a_pool.tile(shape=[128, 512], dtype=F32, space="PSUM", name="a_ps")
            for f in range(512):
                g = f % G
                nc.tensor.matmul(a_ps[32 * g:32 * g + N_M, 0:NB], OH[:, :, f], V[:, :, f],
                                 start=(f < G), stop=(f >= 512 - G),
                                 tile_position=(0, 32 * g), skip_group_check=True)
            # ScalarE runs ONLY these copies, so its scatter-done wait cannot
            # head-of-line-block any prepare work
            ns.copy(out_sb[:, ci, :], a_ps[:, 0:NB])

        state = None
        for ci in range(n_ch + 1):
            nxt = emit_prepare(ci) if ci < n_ch else None
            if state is not None:
                emit_scatter(state)
            state = nxt

        ch_ctx.close()
        nc.sync.dma_start(out, out_sb.rearrange("p c d -> p (c d)"))
        ctx.close()

    nc.compile()
    return nc


_CACHE: dict = {}
LAST_RESULT = None


def _get_nc(n_pairs=6):
    key = n_pairs
    if key not in _CACHE:
        _CACHE[key] = build(n_pairs)
    return _CACHE[key]


def kernel(pred: np.ndarray, target: np.ndarray) -> np.ndarray:
    global LAST_RESULT
    pred = np.ascontiguousarray(pred, dtype=np.float32)
    target = np.ascontiguousarray(target, dtype=np.float32)
    assert pred.shape == (16, 3, 512, 512) and target.shape == (16, 3, 512, 512)

    nc = _get_nc(6)
    mh_buf = make_mh(2.0)
    in_maps = []
    for i in range(N_CORES):
        in_maps.append({
            "pred": pred[2 * i:2 * i + 2],
            "target": target[2 * i:2 * i + 2],
            "mh": mh_buf,
        })
    trace = os.environ.get("KERNEL_TRACE", "0") == "1"
    res = run_bass_kernel_spmd(nc, in_maps, core_ids=list(range(N_CORES)), trace=trace)
    LAST_RESULT = res

    R3C = make_r3c().reshape(N_M * NB, 256)         # [(m,d), k] f64
    losses = []
    for i in range(N_CORES):
        raw = res.results[i]["out"].astype(np.float64).reshape(128, 12, NB)
        # combine the 4 tile_position column-strips: A[ch, m, d]
        A = sum(raw[32 * g:32 * g + N_M] for g in range(4))   # [N_M, 12, NB]
        A = A.transpose(1, 0, 2).reshape(12, N_M * NB)
        Cn = A @ R3C                                          # [12, 256]
        for p in range(6):
            Cp, Ct = Cn[p], Cn[p + 6]
            losses.append(np.mean(np.abs(Cp / Cp[-1] - Ct / Ct[-1])))
    return np.float32(np.mean(losses))


# revision 35
# speedup vs baseline: 1.1900x; 1.0177x over previous
"""Trainium2 Bass kernel for nn_CDFL1HistogramLoss (CDF-L1 histogram loss).

Math (derived from the reference):
  1. jax.image.resize(bilinear, 512->256, antialiased) is a separable 4-tap
     filter: interior out[i] = (x[2i-1] + 3x[2i] + 3x[2i+1] + x[2i+2])/8,
     edges [3,3,1]/7.  Vertical via PE matmul against a constant 512x256
     band matrix (pre-scaled so downstream values are in u/16 units, and
     streamed in even/odd-split column order so the horizontal filter runs
     on contiguous bf16 slices at DVE 2x rate).
  2. The loss needs T(t) = sum_p sigmoid(c*(u_p - t)) for t = 0..256,
     u = 256*x_resized, c = SIGMA/256.  Bucket pixels by h = round(u/16)
     (17 buckets); within a bucket, sigmoid(c*(16m + 8*wn - t)) as a
     function of the normalized offset wn in [-1,1] is approximated to
     ~6e-3 by a degree-14 polynomial (pole of sigmoid at pi*i/c limits the
     Chebyshev rate; deg 14 suffices).  The approximation error is a fixed
     smooth function of u, so it cancels between the pred and target CDFs.
  3. Device computes per-bucket sums of 15 bounded basis polynomials
     B = {1, x, y..y^7, xy..xy^6}, y = 2x^2-1 (all values in [-1,1]; evens
     are Chebyshev T_2k, so bf16 storage stays well-conditioned):
     A[m, d] = sum_{p in bucket m} B_d(wn_p), via a one-hot matmul scatter
     (one-hot over buckets = weights, basis columns = rhs), PSUM-accumulated
     in 4 column-strips (tile_position) over the 512 pixel columns.
  4. Host maps A -> CDF numerators with an f64 least-squares-fit linear map
     and averages the 48 channel losses.

Sharding: data-parallel over batch N: core i handles batches [2i, 2i+1] of
both pred and target (12 channel-histograms, 6 pred/target pairs per core).
"""
import os
import numpy as np

import concourse.bass as bass
import concourse.bacc as bacc
import concourse.mybir as mybir
from concourse import tile
from concourse.bass_utils import run_bass_kernel_spmd

F32 = mybir.dt.float32
BF16 = mybir.dt.bfloat16
I32 = mybir.dt.int32
ALU = mybir.AluOpType
ACT = mybir.ActivationFunctionType

N_CORES = 8
BINS = 256
SIGMA = 300.0
C = SIGMA / BINS          # 1.171875
N_M = 17                  # coarse buckets h = round(u/16) in [0, 16]
NB = 15                   # basis columns {1, x, y..y^7, x*y..x*y^6}


def make_mh(scale: float = 1.0) -> np.ndarray:
    """[512, 256] vertical resize matrix (jax bilinear antialiased 2x down)."""
    M = np.zeros((512, 256), dtype=np.float64)
    for i in range(256):
        if i == 0:
            M[0, 0], M[1, 0], M[2, 0] = 3 / 7, 3 / 7, 1 / 7
        elif i == 255:
            M[509, 255], M[510, 255], M[511, 255] = 1 / 7, 3 / 7, 3 / 7
        else:
            M[2 * i - 1, i] = 1 / 8
            M[2 * i, i] = 3 / 8
            M[2 * i + 1, i] = 3 / 8
            M[2 * i + 2, i] = 1 / 8
    return (scale * M).astype(np.float32)


def _basis_rows(w: np.ndarray) -> np.ndarray:
    """Device basis values at offsets w in [-1,1]: [..., NB] f64."""
    y = 2.0 * w * w - 1.0
    cols = [np.ones_like(w), w, y, y**2, y**3, y**4, y**5, y**6, y**7,
            w * y, w * y**2, w * y**3, w * y**4, w * y**5, w * y**6]
    return np.stack(cols, axis=-1)


def make_r3c() -> np.ndarray:
    """R3C[m, d, k]: maps bucket-basis sums A[m, d] to CDF numerators
    C[k] = T(0) - T(k+1), k = 0..255, via f64 least-squares fits of
    sigmoid(c*(16m + 8w - t)) in the device basis over w in [-1, 1]."""
    wg = np.cos(np.pi * (np.arange(400) + 0.5) / 400)
    B = _basis_rows(wg)                       # [400, NB]
    tg = np.arange(257.0)
    R3 = np.zeros((N_M, NB, 257))
    for m in range(N_M):
        f = 1.0 / (1.0 + np.exp(-C * (16.0 * m + 8.0 * wg[:, None] - tg[None, :])))
        cf, *_ = np.linalg.lstsq(B, f, rcond=None)   # [NB, 257]
        R3[m] = cf
    return R3[:, :, 0:1] - R3[:, :, 1:257]    # [N_M, NB, 256]


def _nonzero_blocks(MH):
    """Which (half, q) 128x128 blocks of MH are nonzero."""
    blocks = {}
    for half in range(2):
        qs = []
        for q in range(4):
            blk = MH[128 * q:128 * (q + 1), 128 * half:128 * (half + 1)]
            if np.any(blk != 0):
                qs.append(q)
        blocks[half] = qs
    return blocks


def build(n_pairs: int = 6):
    """Build the per-core Bass program. Channels: n_pairs pred + n_pairs target."""
    MHs = make_mh(2.0)               # hs = 2*v so that up = 3*s + t is u/16
    mh_blocks = _nonzero_blocks(MHs)
    n_ch = 2 * n_pairs

    nc = bacc.Bacc("TRN2", target_bir_lowering=False, debug=False, num_devices=N_CORES)
    pred = nc.dram_tensor("pred", [2, 3, 512, 512], F32, kind="ExternalInput").ap()
    target = nc.dram_tensor("target", [2, 3, 512, 512], F32, kind="ExternalInput").ap()
    mh = nc.dram_tensor("mh", [512, 256], F32, kind="ExternalInput").ap()
    out = nc.dram_tensor("out", [128, n_ch * NB], F32, kind="ExternalOutput").ap()

    with tile.TileContext(nc) as tc:
        from contextlib import ExitStack
        nv = nc.vector
        ns = nc.scalar
        ctx = ExitStack()
        cpool = ctx.enter_context(tc.tile_pool(name="consts", bufs=1))

        mh_sb = cpool.tile(shape=[128, 4, 256], dtype=F32, name="mh_sb")
        nc.sync.dma_start(mh_sb, mh.rearrange("(q p) w -> p q w", p=128))
        mhb = cpool.tile(shape=[128, 4, 256], dtype=BF16, name="mhb")
        nv.tensor_copy(mhb, mh_sb)
        out_sb = cpool.tile(shape=[128, n_ch, NB], dtype=F32, name="out_sb")

        ch_ctx = ExitStack()
        io_pool = ch_ctx.enter_context(tc.tile_pool(name="io", bufs=2))
        hp_pool = ch_ctx.enter_context(tc.tile_pool(name="hp", bufs=2, space="PSUM"))
        wk_pool = ch_ctx.enter_context(tc.tile_pool(name="wk", bufs=2))
        oh_pool = ch_ctx.enter_context(tc.tile_pool(name="oh", bufs=2))
        a_pool = ch_ctx.enter_context(tc.tile_pool(name="a", bufs=2, space="PSUM"))

        chans = [("p", pi) for pi in range(n_pairs)] + [("t", pi) for pi in range(n_pairs)]

        def emit_prepare(ci):
            grp, pi = chans[ci]
            b, cch = divmod(pi, 3)
            src = (pred if grp == "p" else target)[b, cch]   # [512, 512] dram
            raw = io_pool.tile(shape=[128, 4, 512], dtype=F32, name="raw")
            srcq = src.rearrange("(q p) w -> q p w", p=128)
            for q in range(4):      # per-q chunks so the cast starts sooner
                nc.sync.dma_start(raw[:, q, :], srcq[q])
            rawb = io_pool.tile(shape=[128, 4, 512], dtype=BF16, name="rawb")
            nv.tensor_copy(rawb, raw)   # bf16 weights+rhs enable PE fast weight load

            hs = wk_pool.tile(shape=[128, 2, 512], dtype=BF16, name="hs")
            for half in range(2):
                hp = hp_pool.tile(shape=[128, 512], dtype=F32, space="PSUM", name="hp")
                qs = mh_blocks[half]
                # rhs streamed in even/odd-split order -> hp = [ev(256) | od(256)]
                for qi, q in enumerate(qs):
                    rq = (rawb[:, q, :]
                          .rearrange("p (h two) -> p h two", two=2)
                          .rearrange("p h two -> p two h"))
                    nc.tensor.matmul(
                        hp, mhb[:, q, 128 * half:128 * (half + 1)], rq,
                        start=(qi == 0), stop=(qi == len(qs) - 1),
                    )
                ns.copy(hs[:, half], hp)

            # horizontal 4-tap in u/16 units: up[i] = 3*(ev[i]+od[i]) + od[i-1]+ev[i+1]
            s = wk_pool.tile(shape=[128, 2, 256], dtype=BF16, name="s")
            t = wk_pool.tile(shape=[128, 2, 256], dtype=BF16, name="t")
            up = wk_pool.tile(shape=[128, 2, 256], dtype=BF16, name="up")
            te = wk_pool.tile(shape=[128, 4], dtype=BF16, name="te")
            for half in range(2):
                ev, od = hs[:, half, 0:256], hs[:, half, 256:512]
                nv.tensor_tensor(s[:, half], ev, od, ALU.add)
                nv.tensor_tensor(t[:, half, 1:255], od[:, 0:254], ev[:, 2:256], ALU.add)
                # edges: up[0] = (24/7)s[0] + (8/7)ev[1]; up[255] = (24/7)s[255] + (8/7)od[254]
                nv.tensor_scalar(te[:, 2 * half:2 * half + 1], ev[:, 1:2], 8.0 / 7.0, None, ALU.mult)
                nv.tensor_scalar(te[:, 2 * half + 1:2 * half + 2], od[:, 254:255], 8.0 / 7.0, None, ALU.mult)
                nv.scalar_tensor_tensor(t[:, half, 0:1], s[:, half, 0:1], 3.0 / 7.0,
                                        te[:, 2 * half:2 * half + 1], ALU.mult, ALU.add)
                nv.scalar_tensor_tensor(t[:, half, 255:256], s[:, half, 255:256], 3.0 / 7.0,
                                        te[:, 2 * half + 1:2 * half + 2], ALU.mult, ALU.add)
                nv.scalar_tensor_tensor(up[:, half], s[:, half], 3.0, t[:, half], ALU.mult, ALU.add)

            upf = up.rearrange("p h i -> p (h i)")          # [128, 512], u/16 in [0,16]
            hr = wk_pool.tile(shape=[128, 512], dtype=BF16, name="hr")
            h2 = wk_pool.tile(shape=[128, 512], dtype=BF16, name="h2")
            V = oh_pool.tile(shape=[128, NB, 512], dtype=BF16, name="V")
            OH = oh_pool.tile(shape=[128, N_M, 512], dtype=BF16, name="OH")
            if ci < 2:
                nc.gpsimd.memset(V[:, 0, :], 1.0)           # ones plane, per physical buf

            # channel 0 is the pipeline ramp: emit its index/basis/one-hot
            # chain in two f-chunks so the first scatter can start early
            for lo, hi in ([(0, 256), (256, 512)] if ci == 0 else [(0, 512)]):
                uc = upf[:, lo:hi]
                # h = round(up) via bf16 magic constant: up+128 lands on the
                # integer grid (ulp=1 in [128,256)), rounding to nearest int.
                nv.tensor_scalar(hr[:, lo:hi], uc, 128.0, None, ALU.add)
                nv.tensor_scalar(h2[:, lo:hi], hr[:, lo:hi], 2.0, -256.0, ALU.mult, ALU.add)
                x = V[:, 1, lo:hi]
                nv.scalar_tensor_tensor(x, uc, 2.0, h2[:, lo:hi], ALU.mult, ALU.subtract)
                wnsq = wk_pool.tile(shape=[128, 512], dtype=BF16, name="wnsq")
                nv.tensor_tensor(wnsq[:, lo:hi], x, x, ALU.mult)
                y = V[:, 2, lo:hi]
                nv.tensor_scalar(y, wnsq[:, lo:hi], 2.0, -1.0, ALU.mult, ALU.add)
                c = lambda d: V[:, d, lo:hi]
                nv.tensor_tensor(c(3), y, y, ALU.mult)            # y2
                nv.tensor_tensor(c(4), c(3), y, ALU.mult)         # y3
                nv.tensor_tensor(c(5), c(3), c(3), ALU.mult)      # y4
                nv.tensor_tensor(c(6), c(5), y, ALU.mult)         # y5
                nv.tensor_tensor(c(7), c(5), c(3), ALU.mult)      # y6
                nv.tensor_tensor(c(8), c(5), c(4), ALU.mult)      # y7
                nv.tensor_tensor(c(9), x, y, ALU.mult)            # xy
                nv.tensor_tensor(c(10), x, c(3), ALU.mult)        # xy2
                nv.tensor_tensor(c(11), c(10), y, ALU.mult)       # xy3
                nv.tensor_tensor(c(12), x, c(5), ALU.mult)        # xy4
                nv.tensor_tensor(c(13), c(12), y, ALU.mult)       # xy5
                nv.tensor_tensor(c(14), c(12), c(3), ALU.mult)    # xy6
                for m in range(N_M):
                    nv.tensor_scalar(OH[:, m, lo:hi], h2[:, lo:hi], float(2 * m), None, ALU.is_equal)
            return (ci, OH, V)

        def emit_scatter(st):
            ci, OH, V = st
            G = 4
            a_ps = a_pool.tile(shape=[128, 512], dtype=F32, space="PSUM", name="a_ps")
            for f in range(512):
                g = f % G
                nc.tensor.matmul(a_ps[32 * g:32 * g + N_M, 0:NB], OH[:, :, f], V[:, :, f],
                                 start=(f < G), stop=(f >= 512 - G),
                                 tile_position=(0, 32 * g), skip_group_check=True)
            # ScalarE runs ONLY these copies, so its scatter-done wait cannot
            # head-of-line-block any prepare work
            ns.copy(out_sb[:, ci, :], a_ps[:, 0:NB])

        state = None
        for ci in range(n_ch + 1):
            nxt = emit_prepare(ci) if ci < n_ch else None
            if state is not None:
                emit_scatter(state)
            state = nxt

        ch_ctx.close()
        nc.sync.dma_start(out, out_sb.rearrange("p c d -> p (c d)"))
        ctx.close()

    nc.compile()
    return nc


_CACHE: dict = {}
LAST_RESULT = None


def _get_nc(n_pairs=6):
    key = n_pairs
    if key not in _CACHE:
        _CACHE[key] = build(n_pairs)
    return _CACHE[key]


def kernel(pred: np.ndarray, target: np.ndarray) -> np.ndarray:
    global LAST_RESULT
    pred = np.ascontiguousarray(pred, dtype=np.float32)
    target = np.ascontiguousarray(target, dtype=np.float32)
    assert pred.shape == (16, 3, 512, 512) and target.shape == (16, 3, 512, 512)

    nc = _get_nc(6)
    mh_buf = make_mh(2.0)
    in_maps = []
    for i in range(N_CORES):
        in_maps.append({
            "pred": pred[2 * i:2 * i + 2],
            "target": target[2 * i:2 * i + 2],
            "mh": mh_buf,
        })
    trace = os.environ.get("KERNEL_TRACE", "0") == "1"
    res = run_bass_kernel_spmd(nc, in_maps, core_ids=list(range(N_CORES)), trace=trace)
    LAST_RESULT = res

    R3C = make_r3c().reshape(N_M * NB, 256)         # [(m,d), k] f64
    losses = []
    for i in range(N_CORES):
        raw = res.results[i]["out"].astype(np.float64).reshape(128, 12, NB)
        # combine the 4 tile_position column-strips: A[ch, m, d]
        A = sum(raw[32 * g:32 * g + N_M] for g in range(4))   # [N_M, 12, NB]
        A = A.transpose(1, 0, 2).reshape(12, N_M * NB)
        Cn = A @ R3C                                          # [12, 256]
        for p in range(6):
            Cp, Ct = Cn[p], Cn[p + 6]
            losses.append(np.mean(np.abs(Cp / Cp[-1] - Ct / Ct[-1])))
    return np.float32(np.mean(losses))
